# revision 4
# baseline (speedup 1.0000x reference)
"""DILATE divergence loss (soft-DTW divergence + temporal path loss) on 8 Trainium2
NeuronCores, data-parallel over the batch.

Math (per problem, in scaled units X' = X/gamma):
  forward hard pass :  O[i,j] = D[i,j] + min(O[i-1,j-1], O[i-1,j], O[i,j-1])
  forward soft pass :  z[i,j] = sum_pred e^{m[i,j]-O_pred} * z_pred, m = O - D
                       (exp-domain affine scan; per-cell offset O keeps args <= 0,
                        max(O-R) ~= 26 << 88 so fp32 is safe)
                       R[i,j] = O[i,j] - ln z[i,j]   (exact soft-DTW)
  backward          :  E[i,j] = sum_succ e^{(R_succ - D_succ) - R[i,j]} * E_succ
  loss = alpha*mean_b gamma*(R_xy - (R_xx+R_yy)/2)[N-1,N-1]
         + (1-alpha) * sum_ij mean_b E_xy[i,j]*(i-j)^2 / N^2

v2 engine layout (vs v1): D-row squares on the Scalar engine (Square act),
pm/t2 on GpSimd, no Ln in the loops (batched O-ln z between passes -> one act
table switch instead of 511), m = O - D identity saves a Vector op, temporal
E*Omega contraction done once on a [128,4096] layout instead of per-row.
Final corners/logs are combined on the host in float64.
"""

import numpy as np

ALPHA = 0.5
GAMMA = 0.01
B, N, DF = 64, 256, 1
NCORES = 8
BPC = B // NCORES          # 8 batches per core
P = 3 * BPC                # 24 stacked DP problems per core
PB = BPC                   # 8 backward problems per core (xy only)
INF = 1.0e9                # scaled-units "infinity"
SQG = float(np.sqrt(GAMMA))
NG = 16                    # row groups for the [128, 16*N] storage layout

_PROGRAM_CACHE = {}


def build_program():
    """Builds the Bass program (one NEFF, run SPMD on 8 cores). Returns nc."""
    import concourse.mybir as mybir
    from concourse import bacc
    from concourse.tile import TileContext

    dt = mybir.dt.float32
    Alu = mybir.AluOpType
    Act = mybir.ActivationFunctionType

    nc = bacc.Bacc("TRN2", target_bir_lowering=False, debug=False, num_devices=NCORES)

    cpk_d = nc.dram_tensor("cpack", [P, 2 * N], dt, kind="ExternalInput").ap()
    om_d = nc.dram_tensor("omega", [128, NG * N], dt, kind="ExternalInput").ap()
    ocor_d = nc.dram_tensor("ocor", [P, 1], dt, kind="ExternalOutput").ap()
    zcor_d = nc.dram_tensor("zcor", [P, 1], dt, kind="ExternalOutput").ap()
    rowdot_d = nc.dram_tensor("rowdot", [128, NG], dt, kind="ExternalOutput").ap()
    ost_d = nc.dram_tensor("ostore", [PB, N, N], dt, kind="Internal").ap()
    zst_d = nc.dram_tensor("zstore", [PB, N, N], dt, kind="Internal").ap()
    rst_d = nc.dram_tensor("rstore", [PB, N, N], dt, kind="Internal").ap()
    est_d = nc.dram_tensor("estore", [PB, N, N], dt, kind="Internal").ap()

    V = nc.vector
    G = nc.gpsimd
    S = nc.scalar

    with TileContext(nc) as tc:
        with (
            tc.tile_pool(name="state", bufs=1) as st,
            tc.tile_pool(name="work", bufs=4) as wk,
        ):
            # ---- constants in ----
            cpk_sb = st.tile([P, 2 * N], dt)
            nc.sync.dma_start(cpk_sb[:], cpk_d[:])
            tneg_sb = cpk_sb[:, 0:N]       # -t/sqrt(g) per problem
            ss_sb = cpk_sb[:, N:2 * N]     # s/sqrt(g)
            om_sb = st.tile([128, NG * N], dt)
            nc.sync.dma_start(om_sb[:], om_d[:])  # consumed only in post-pass

            # ---- forward state: padded row buffers, col 0 = j=-1 pad ----
            obufs = [st.tile([P, N + 1], dt, name=f"obuf{k}", tag=f"obuf{k}") for k in range(2)]
            zbufs = [st.tile([P, N + 1], dt, name=f"zbuf{k}", tag=f"zbuf{k}") for k in range(2)]
            G.memset(obufs[0][:], INF)       # O[-1, j] = INF ...
            G.memset(obufs[0][:, 0:1], 0.0)  # ... except corner R[-1,-1] = 0
            G.memset(obufs[1][:, 0:1], INF)  # left pad of row buffers
            G.memset(zbufs[0][:], 0.0)
            G.memset(zbufs[0][:, 0:1], 1.0)  # corner z = 1
            G.memset(zbufs[1][:, 0:1], 0.0)

            for i in range(N):
                prevO, curO = obufs[i % 2], obufs[(i + 1) % 2]
                prevZ, curZ = zbufs[i % 2], zbufs[(i + 1) % 2]
                if i == 1:
                    # obufs[0]/zbufs[0] stop being the virtual row: fix pads
                    G.memset(curO[:, 0:1], INF)
                    G.memset(curZ[:, 0:1], 0.0)

                # D row on the Scalar engine: dsq = (ss + (-t_i))^2
                dsq = wk.tile([P, N], dt, tag="dsq")
                S.activation(dsq[:], ss_sb[:], Act.Square, bias=tneg_sb[:, i:i + 1])

                # hard pass: pm on Vector (Pool lacks TT min), t2 on GpSimd
                pm = wk.tile([P, N], dt, tag="pm")
                V.tensor_tensor(pm[:], prevO[:, 0:N], prevO[:, 1:N + 1], Alu.min)
                t2 = wk.tile([P, N], dt, tag="t2")
                G.tensor_tensor(t2[:], pm[:], dsq[:], Alu.add)
                V.tensor_tensor_scan(curO[:, 1:N + 1], dsq[:], t2[:], INF,
                                     Alu.add, Alu.min)

                # soft pass: m = O - D (= min of 3 preds); one fused exp
                m = wk.tile([P, N], dt, tag="m")
                V.tensor_tensor(m[:], curO[:, 1:N + 1], dsq[:], Alu.subtract)
                A3 = wk.tile([P, 3 * N], dt, tag="A3")
                V.tensor_tensor(A3[:, 0:N], m[:], prevO[:, 0:N], Alu.subtract)
                V.tensor_tensor(A3[:, N:2 * N], m[:], prevO[:, 1:N + 1], Alu.subtract)
                V.tensor_tensor(A3[:, 2 * N:3 * N], m[:], curO[:, 0:N], Alu.subtract)
                E3 = wk.tile([P, 3 * N], dt, tag="E3")
                S.activation(E3[:], A3[:], Act.Exp)

                p1 = wk.tile([P, N], dt, tag="p1")
                G.tensor_tensor(p1[:], E3[:, 0:N], prevZ[:, 0:N], Alu.mult)
                p2 = wk.tile([P, N], dt, tag="p2")
                V.tensor_tensor(p2[:], E3[:, N:2 * N], prevZ[:, 1:N + 1], Alu.mult)
                prep = wk.tile([P, N], dt, tag="prep")
                V.tensor_tensor(prep[:], p1[:], p2[:], Alu.add)
                V.tensor_tensor_scan(curZ[:, 1:N + 1], E3[:, 2 * N:3 * N], prep[:],
                                     0.0, Alu.mult, Alu.add)

                # store O/z rows (xy problems) for the mid-pass R build
                nc.sync.dma_start(ost_d[:, i, :], curO[0:PB, 1:N + 1])
                nc.sync.dma_start(zst_d[:, i, :], curZ[0:PB, 1:N + 1])

                if i == N - 1:
                    nc.sync.dma_start(ocor_d[:], curO[:, N:N + 1])
                    nc.sync.dma_start(zcor_d[:], curZ[:, N:N + 1])

            # ---- mid-pass: R = O - ln z, batched on [128, NG*N] ----
            z128 = st.tile([128, NG * N], dt)
            o128 = st.tile([128, NG * N], dt)
            for g in range(NG):
                nc.sync.dma_start(z128[8 * g:8 * g + 8, :], zst_d[:, NG * g:NG * g + NG, :])
                nc.sync.dma_start(o128[8 * g:8 * g + 8, :], ost_d[:, NG * g:NG * g + NG, :])
            lnz = st.tile([128, NG * N], dt)
            S.activation(lnz[:], z128[:], Act.Ln)
            V.tensor_tensor(o128[:], o128[:], lnz[:], Alu.subtract)  # o128 := R
            for g in range(NG):
                nc.sync.dma_start(rst_d[:, NG * g:NG * g + NG, :], o128[8 * g:8 * g + 8, :])

            # ---- backward: E rows, right-padded buffers (col N = j=N pad) ----
            ebufs = [st.tile([PB, N + 1], dt, name=f"ebuf{k}", tag=f"ebuf{k}") for k in range(2)]
            ccbufs = [st.tile([PB, N + 1], dt, name=f"ccbuf{k}", tag=f"ccbuf{k}") for k in range(2)]
            G.memset(ebufs[0][:, N:N + 1], 0.0)
            G.memset(ebufs[1][:, N:N + 1], 0.0)
            G.memset(ccbufs[0][:, N:N + 1], -INF)
            G.memset(ccbufs[1][:, N:N + 1], -INF)

            for i in range(N - 1, -1, -1):
                curE, prevE = ebufs[i % 2], ebufs[(i + 1) % 2]
                curC, nextC = ccbufs[i % 2], ccbufs[(i + 1) % 2]

                rr = wk.tile([PB, N], dt, tag="rr")
                nc.sync.dma_start(rr[:], rst_d[:, i, :])
                dsqb = wk.tile([PB, N], dt, tag="dsqb")
                S.activation(dsqb[:], ss_sb[0:PB, :], Act.Square,
                             bias=tneg_sb[0:PB, i:i + 1])
                # cc = R - D (softmin row), right pad -INF
                V.tensor_tensor(curC[:, 0:N], rr[:], dsqb[:], Alu.subtract)

                A3b = wk.tile([PB, 3 * N], dt, tag="A3b")
                W3 = wk.tile([PB, 3 * N], dt, tag="W3")
                # w_l arg: cc[j+1] - R[i,j]
                V.tensor_tensor(A3b[:, 2 * N:3 * N], curC[:, 1:N + 1], rr[:],
                                Alu.subtract)
                prepb = wk.tile([PB, N], dt, tag="prepb")
                if i == N - 1:
                    S.activation(W3[:, 2 * N:3 * N], A3b[:, 2 * N:3 * N], Act.Exp)
                    G.memset(prepb[:], 0.0)
                    G.memset(prepb[:, N - 1:N], 1.0)
                else:
                    V.tensor_tensor(A3b[:, 0:N], nextC[:, 0:N], rr[:], Alu.subtract)
                    V.tensor_tensor(A3b[:, N:2 * N], nextC[:, 1:N + 1], rr[:],
                                    Alu.subtract)
                    S.activation(W3[:], A3b[:], Act.Exp)
                    q1 = wk.tile([PB, N], dt, tag="q1")
                    V.tensor_tensor(q1[:], W3[:, N:2 * N], prevE[:, 1:N + 1], Alu.mult)
                    q2 = wk.tile([PB, N], dt, tag="q2")
                    V.tensor_tensor(q2[:], W3[:, 0:N], prevE[:, 0:N], Alu.mult)
                    V.tensor_tensor(prepb[:], q1[:], q2[:], Alu.add)

                # reverse affine scan: E[i,j] = w_l[j]*E[i,j+1] + prep[j]
                V.tensor_tensor_scan(curE[:, 0:N][:, ::-1],
                                     W3[:, 2 * N:3 * N][:, ::-1],
                                     prepb[:][:, ::-1], 0.0, Alu.mult, Alu.add)
                nc.sync.dma_start(est_d[:, i, :], curE[:, 0:N])

            # ---- post-pass: rowdot[p+8g, r] = sum_j E[p, 16g+r, j]*(i-j)^2 ----
            e128 = st.tile([128, NG * N], dt)
            for g in range(NG):
                nc.sync.dma_start(e128[8 * g:8 * g + 8, :], est_d[:, NG * g:NG * g + NG, :])
            V.tensor_tensor(e128[:], e128[:], om_sb[:], Alu.mult)
            rowdot_sb = st.tile([128, NG], dt)
            for r in range(NG):
                V.tensor_reduce(rowdot_sb[:, r:r + 1], e128[:, N * r:N * (r + 1)],
                                mybir.AxisListType.X, Alu.add)
            nc.sync.dma_start(rowdot_d[:], rowdot_sb[:])

    nc.finalize()
    return nc


def get_program():
    if "nc" not in _PROGRAM_CACHE:
        _PROGRAM_CACHE["nc"] = build_program()
    return _PROGRAM_CACHE["nc"]


def make_in_maps(input, target):
    """Host-side shard prep: per-core input dicts (all fp32 numpy)."""
    x = np.asarray(input, np.float32).reshape(B, N) / SQG   # "input"  -> s of xy
    y = np.asarray(target, np.float32).reshape(B, N) / SQG  # "target" -> t of xy
    # omega[p + 8g, N*r + j] = (i - j)^2 with i = 16g + r  (same for every core)
    ii = np.arange(N, dtype=np.float64)
    omf = (ii[:, None] - ii[None, :]) ** 2          # [i, j]
    om = np.zeros((128, NG * N), np.float32)
    for g in range(NG):
        blk = omf[NG * g:NG * g + NG, :].reshape(1, NG * N)  # rows 16g..16g+15
        om[8 * g:8 * g + 8, :] = blk
    in_maps = []
    for c in range(NCORES):
        sl = slice(c * BPC, (c + 1) * BPC)
        t24 = np.concatenate([y[sl], y[sl], x[sl]], axis=0).astype(np.float32)
        s24 = np.concatenate([x[sl], y[sl], x[sl]], axis=0).astype(np.float32)
        cpk = np.zeros((P, 2 * N), np.float32)
        cpk[:, 0:N] = -t24
        cpk[:, N:2 * N] = s24
        in_maps.append({"cpack": cpk, "omega": om})
    return in_maps


def combine_outputs(results):
    """results: per-core {'ocor':[24,1],'zcor':[24,1],'rowdot':[128,16]} -> loss."""
    shape_terms = []
    tacc_total = 0.0
    for r in results:
        oc = np.asarray(r["ocor"], np.float64).reshape(P)
        zc = np.asarray(r["zcor"], np.float64).reshape(P)
        fin = oc - np.log(np.maximum(zc, 1e-300))          # R'[N-1,N-1] scaled
        xy, xx, yy = fin[0:BPC], fin[BPC:2 * BPC], fin[2 * BPC:3 * BPC]
        shape_terms.append(GAMMA * (xy - 0.5 * (xx + yy)))
        tacc_total += float(np.asarray(r["rowdot"], np.float64).sum())
    loss_shape = float(np.mean(np.concatenate(shape_terms)))
    loss_temporal = tacc_total / (B * N * N)
    return np.float32(ALPHA * loss_shape + (1.0 - ALPHA) * loss_temporal)


def kernel(input, target):
    from concourse import bass_utils
    nc = get_program()
    in_maps = make_in_maps(input, target)
    res = bass_utils.run_bass_kernel_spmd(nc, in_maps, core_ids=list(range(NCORES)))
    return combine_outputs(res.results)


if __name__ == "__main__":
    rng = np.random.default_rng(0)
    inp = rng.standard_normal((B, N, DF), np.float32)
    tgt = rng.standard_normal((B, N, DF), np.float32)
    print("loss:", kernel(input=inp, target=tgt))
